# revision 23
# baseline (speedup 1.0000x reference)
"""Multi-head self-attention (B=2, S=2048, D=1024, H=16) on 8 Trainium2 NeuronCores.

Sharding: batch x head-group. Core c = b*4 + g handles batch b and heads 4g..4g+3
(Megatron-style TP: Wq/Wk/Wv column-sharded, Wo row-sharded; partial outputs
summed on the host).

v3: bf16 matmuls, phase-pipelined attention. The 8 (n, hp) streams are
processed one per phase; in phase k the PE computes scores for stream k while
ctx / denominator matmuls consume stream k-1's exp tiles (held in a ring), so
the scalar engine (exp: the roofline engine at ~135us) is continuously fed and
the PE stays dense (keeps the HAM clock at 2.4 GHz). Projections and the
output projection are interleaved as per-phase fillers on their own PSUM bank.

Denominators: 4 M=1 ones-stationary col tiles (psum parts 0/32/64/96 = both
heads of both hp streams of one n) share a single 512-cycle PE slot per kc.
Normalization: denom rows -> DRAM bounce -> partition-broadcast [128,512] ->
reciprocal_approx_fast -> fused scalar_tensor_tensor (ctx * 1/den -> bf16).

PSUM: scores ring 2x[128,2,512] (4 banks) + ctx accumulators (2) + denom (1)
+ proj/out shared bank (1) = 8.
"""
import sys

sys.path.insert(0, "/opt/trn_rl_repo")

import numpy as np
import ml_dtypes

import concourse.bass as bass
import concourse.tile as tile
from concourse import bacc, mybir
from concourse.bass_utils import run_bass_kernel_spmd

F32 = mybir.dt.float32
BF16 = mybir.dt.bfloat16
NP_BF16 = ml_dtypes.bfloat16

S = 2048          # sequence length per batch
D = 1024          # embedding dim
HG = 4            # heads per core
HD = 64           # head dim
GC = HG * HD      # group cols = 256
P = 128
NQ = 4            # q chunks of 512
QW = 512          # q chunk width
NKC = 16          # key-position chunks of 128
KO = 8            # contraction chunks of 128 over D

_NC_CACHE = {}


def _build():
    if "nc" in _NC_CACHE:
        return _NC_CACHE["nc"]
    nc = bacc.Bacc(trn_type="TRN2", target_bir_lowering=False, debug=False)
    xt_d = nc.dram_tensor("xt", [D, S], BF16, kind="ExternalInput")
    wq_d = nc.dram_tensor("wq", [D, GC], BF16, kind="ExternalInput")
    wk_d = nc.dram_tensor("wk", [D, GC], BF16, kind="ExternalInput")
    wv_d = nc.dram_tensor("wv", [D, GC], BF16, kind="ExternalInput")
    wo_d = nc.dram_tensor("wo", [GC, D], BF16, kind="ExternalInput")
    out_d = nc.dram_tensor("out_t", [D, S], F32, kind="ExternalOutput")
    scr_d = nc.dram_tensor("nrm_scratch", [NQ, 2, 2, QW], F32)
    with tile.TileContext(nc) as tc:
        _emit(nc, tc, xt_d, wq_d, wk_d, wv_d, wo_d, out_d, scr_d)
    nc.compile()
    _dedup_ldweights(nc)
    _NC_CACHE["nc"] = nc
    return nc


def _dedup_ldweights(nc):
    """Drop an InstLdweights identical to the previous one when only
    (non-weight-changing) InstMatmult sit between them: the PE array keeps
    its weights, so the reload is redundant."""
    removed = 0
    for blk in nc.main_func.blocks:
        keep = []
        last = None
        for ins in blk.instructions:
            cn = type(ins).__name__
            if cn == "InstLdweights":
                si = ins.sync_info
                clean = si is None or (not si.on_wait and not si.on_update)

                def _apkey(a):
                    return (getattr(a, "memref", None),
                            getattr(a, "memsetref", None),
                            getattr(a, "offset", None),
                            str(getattr(a, "ap", None)),
                            str(getattr(a, "dtype", None)))
                key = (tuple(_apkey(a) for a in ins.ins),
                       str(getattr(ins, "tile_position", None)),
                       str(getattr(ins, "perf_mode", None)),
                       str(getattr(ins, "is_transpose", None)))
                if clean and last is not None and key == last:
                    removed += 1
                    continue
                last = key
            elif cn != "InstMatmult":
                if getattr(ins, "engine", None) is not None and                         str(getattr(ins, "engine")) == "EngineType.PE":
                    last = None
            keep.append(ins)
        blk.instructions[:] = keep
    return removed


def _emit(nc, tc, xt_d, wq_d, wk_d, wv_d, wo_d, out_d, scr_d):
    with tc.tile_pool(name="big", bufs=1) as big, \
         tc.tile_pool(name="expool", bufs=22) as expool, \
         tc.tile_pool(name="evac", bufs=4) as evac, \
         tc.tile_pool(name="nrm", bufs=4) as nrm, \
         tc.tile_pool(name="ps_sp", bufs=2, space="PSUM") as ps_sp, \
         tc.tile_pool(name="ps_ctx", bufs=2, space="PSUM") as ps_ctx, \
         tc.tile_pool(name="ps_den", bufs=1, space="PSUM") as ps_den, \
         tc.tile_pool(name="ps_o", bufs=1, space="PSUM") as ps_o:
        # ---- persistent SBUF tensors ----
        qt = big.tile([P, 2, S], BF16)        # head h at parts (h%2)*64, chunk h//2
        kt = big.tile([P, 2, S], BF16)
        va2 = big.tile([P, 2, NKC, P], BF16)  # [kpart, hp, kc, head-even|head-odd]
        ct = big.tile([P, 2, S], BF16)        # normalized ctxT, same layout as qt
        wo_sb = big.tile([P, 2, D], BF16)
        ones_w = big.tile([P, 1], BF16)       # denominator stationary
        xs = big.tile([P, KO, S], BF16)       # x.T, [d_in(128) x ko x s]
        wq = big.tile([P, KO, GC], BF16)
        wk = big.tile([P, KO, GC], BF16)
        wv = big.tile([P, KO, GC], BF16)

        nc.vector.memset(ones_w[:].bitcast(mybir.dt.uint16), 0x3F80)
        # DMA order drives time-to-first-exp: wk/wq first, then the n0 slice
        # of x, then wv (P1's V combos), the rest of x, wo last. Small pieces
        # spread across the 16 DMA queues (~22 GB/s each).
        for h in range(2):
            ks = slice(h * 4, h * 4 + 4)
            nc.sync.dma_start(
                wk[:, ks, :],
                wk_d[h * D // 2:(h + 1) * D // 2, :].rearrange(
                    "(ko p) m -> p ko m", p=P))
        for ko in range(KO):
            nc.sync.dma_start(xs[:, ko, 0:QW], xt_d[ko * P:(ko + 1) * P, 0:QW])
        for h in range(2):
            ks = slice(h * 4, h * 4 + 4)
            nc.sync.dma_start(
                wq[:, ks, :],
                wq_d[h * D // 2:(h + 1) * D // 2, :].rearrange(
                    "(ko p) m -> p ko m", p=P))
        for h in range(2):
            ks = slice(h * 4, h * 4 + 4)
            nc.sync.dma_start(
                wv[:, ks, :],
                wv_d[h * D // 2:(h + 1) * D // 2, :].rearrange(
                    "(ko p) m -> p ko m", p=P))
        for n in range(1, NQ):
            for ko in range(KO):
                nc.sync.dma_start(xs[:, ko, n * QW:(n + 1) * QW],
                                  xt_d[ko * P:(ko + 1) * P, n * QW:(n + 1) * QW])
        for c in range(4):
            nc.sync.dma_start(
                wo_sb[:, c // 2, (c % 2) * QW:(c % 2 + 1) * QW],
                wo_d[(c // 2) * P:(c // 2 + 1) * P,
                     (c % 2) * QW:(c % 2 + 1) * QW])

        def proj_combo(w_sb, dst, m, n):
            """dst[:, m, n*QW:+QW] = (w_sb chunk).T @ xs chunk, K=128 x 8."""
            pp = ps_sp.tile([P, 2, QW], F32, tag="sp")
            for ko in range(KO):
                nc.tensor.matmul(pp[:, 0, :],
                                 w_sb[:, ko, m * P:(m + 1) * P],
                                 xs[:, ko, n * QW:(n + 1) * QW],
                                 start=(ko == 0), stop=(ko == KO - 1))
            nc.vector.tensor_copy(dst[:, m, n * QW:(n + 1) * QW], pp[:, 0, :])

        def v_combo(sc):
            """va2[:, :, sc, :] = V rows sc*128..+128 (natural layout)."""
            pp = ps_sp.tile([P, 2, QW], F32, tag="sp")
            for ko in range(KO):
                nc.tensor.matmul(pp[:, 0, 0:GC],
                                 xs[:, ko, sc * P:(sc + 1) * P],
                                 wv[:, ko, :],
                                 start=(ko == 0), stop=(ko == KO - 1))
            nc.vector.tensor_copy(
                va2[:, :, sc, :],
                pp[:, 0, 0:GC].rearrange("p (h c) -> p h c", c=P))

        def scores_exp(hp, n, kc):
            """Returns ex tile [P, 2, QW] bf16 = exp(scoresT/8) for both heads."""
            sp = ps_sp.tile([P, 2, QW], F32, tag="sp")
            for e in range(2):
                lo = e * 64
                nc.tensor.matmul(
                    sp[:, e, :],
                    kt[lo:lo + 64, hp, kc * P:(kc + 1) * P],
                    qt[lo:lo + 64, hp, n * QW:(n + 1) * QW],
                    start=True, stop=True)
            ex = expool.tile([P, 2, QW], BF16, tag="ex")
            nc.scalar.activation(
                ex[:].rearrange("p a b -> p (a b)"),
                sp[:].rearrange("p a b -> p (a b)"),
                mybir.ActivationFunctionType.Exp,
                scale=0.125)
            return ex

        def ctx_pair(hp, kc, ex, ctx_ps):
            first, last = kc == 0, kc == NKC - 1
            nc.tensor.matmul(ctx_ps[0:64, :], va2[:, hp, kc, 0:64],
                             ex[:, 0, :], start=first, stop=last,
                             tile_position=(0, 0))
            nc.tensor.matmul(ctx_ps[64:128, :], va2[:, hp, kc, 64:128],
                             ex[:, 1, :], start=first, stop=last,
                             tile_position=(0, 64))

        def den4(kc, ex0, ex1, den_ps):
            """One slot: denominators for both heads of both hp streams of n."""
            first, last = kc == 0, kc == NKC - 1
            for j, exs in enumerate((ex0[:, 0, :], ex0[:, 1, :],
                                     ex1[:, 0, :], ex1[:, 1, :])):
                c = 32 * j
                nc.tensor.matmul(den_ps[c:c + 1, :], ones_w[:, 0:1], exs,
                                 start=first, stop=last, tile_position=(0, c))

        def den_chain_io(n, hp, den_ps):
            """DRAM bounce for (n, hp) denoms -> bc broadcast tile (no recip)."""
            base = 64 * hp
            dsb = nrm.tile([P, QW], F32, tag="dsb")
            for e in range(2):
                r = base + 32 * e
                nc.vector.tensor_copy(dsb[r:r + 1, :], den_ps[r:r + 1, :])
            for e in range(2):
                r = base + 32 * e
                sl = scr_d[n, hp, e]
                nc.sync.dma_start(sl.unsqueeze(0), dsb[r:r + 1, :])
            bc = nrm.tile([P, QW], F32, tag="bc")
            for e in range(2):
                sl = scr_d[n, hp, e]
                bc_src = bass.AP(tensor=sl.tensor, offset=sl.offset,
                                 ap=[[0, 64]] + list(sl.ap))
                nc.sync.dma_start(bc[64 * e:64 * (e + 1), :], bc_src)
            return bc

        def den_chain(n, hp, den_ps):
            bc = den_chain_io(n, hp, den_ps)
            nc.vector.reciprocal_approx_fast(bc[:], bc[:])
            return bc

        def norm_apply(n, hp, ctx_ps, bc):
            ns = slice(n * QW, (n + 1) * QW)
            nc.vector.scalar_tensor_tensor(
                ct[:, hp, ns], ctx_ps[:], 1.0, bc[:],
                mybir.AluOpType.mult, mybir.AluOpType.mult)

        def normalize(n, hp, ctx_ps, den_ps):
            norm_apply(n, hp, ctx_ps, den_chain(n, hp, den_ps))

        def out_chunk(n, m, pool_sp=False):
            ns = slice(n * QW, (n + 1) * QW)
            if pool_sp:
                pot = ps_sp.tile([P, 2, QW], F32, tag="sp", name=f"po_sp_{n}_{m}")
                po = pot[:, 0, :]
            else:
                pot = ps_o.tile([P, QW], F32, tag="po", name=f"po_{n}_{m}")
                po = pot[:]
            nc.tensor.matmul(po, wo_sb[:, 0, m * P:(m + 1) * P],
                             ct[:, 0, ns], start=True, stop=False)
            nc.tensor.matmul(po, wo_sb[:, 1, m * P:(m + 1) * P],
                             ct[:, 1, ns], start=False, stop=True)
            ot = evac.tile([P, QW], F32, tag="ot")
            nc.vector.tensor_copy(ot[:], po)
            nc.sync.dma_start(out_d[m * P:(m + 1) * P, ns], ot[:])

        def proj_halves(w_sb, dst, m, n):
            """proj_combo split in two filler units (smaller PE injections)."""
            cell = {}

            def f1():
                cell["pp"] = ps_sp.tile([P, 2, QW], F32, tag="sp", name=f"ph_{id(cell)}")
                for ko in range(4):
                    nc.tensor.matmul(cell["pp"][:, 0, :],
                                     w_sb[:, ko, m * P:(m + 1) * P],
                                     xs[:, ko, n * QW:(n + 1) * QW],
                                     start=(ko == 0), stop=False)

            def f2():
                pp = cell["pp"]
                for ko in range(4, KO):
                    nc.tensor.matmul(pp[:, 0, :],
                                     w_sb[:, ko, m * P:(m + 1) * P],
                                     xs[:, ko, n * QW:(n + 1) * QW],
                                     start=False, stop=(ko == KO - 1))
                nc.vector.tensor_copy(dst[:, m, n * QW:(n + 1) * QW],
                                      pp[:, 0, :])

            return [f1, f2]

        # ---- schedule: emission order == per-engine execution order ----
        streams = [(n, hp) for n in range(NQ) for hp in range(2)]

        # PE clock warm-up: dummy matmuls while the input DMAs run, so the
        # HAM un-throttles the PE (1.2 -> 2.4 GHz) before real work.
        dmy_ps = ps_o.tile([P, QW], F32, tag="po", name="dmy_ps")
        for _ in range(240):
            nc.tensor.matmul(dmy_ps[0:1, 0:1], ones_w[:, 0:1], ones_w[:, 0:1],
                             start=True, stop=True)

        # P0 (dense warm-up): minimum to unblock stream 0 = (n0, h0)
        proj_combo(wk, kt, 0, 0)
        proj_combo(wq, qt, 0, 0)

        # per-phase fillers; pops-per-kc in fillers_rate
        fillers = [[] for _ in range(8)]
        # P1: V (ctx(s0) needs it in P2), rest of kt, qt(m1, n0) for s1.
        # KT m1 / QT m1n0 are needed at P2 kc0 -> interleave them early.
        p1 = []
        extras = [lambda: proj_combo(wk, kt, 0, 1),
                  lambda: proj_combo(wk, kt, 1, 0),
                  lambda: proj_combo(wk, kt, 0, 2),
                  lambda: proj_combo(wk, kt, 1, 1),
                  lambda: proj_combo(wk, kt, 0, 3),
                  lambda: proj_combo(wk, kt, 1, 2),
                  lambda: proj_combo(wk, kt, 1, 3),
                  lambda: proj_combo(wq, qt, 1, 0)]
        for sc in range(NKC):
            p1.append(lambda sc=sc: v_combo(sc))
            if sc < len(extras):
                p1.append(extras[sc])
        fillers[0] = p1
        fillers[1] = proj_halves(wq, qt, 0, 1) + proj_halves(wq, qt, 1, 1)
        fillers[2] = proj_halves(wq, qt, 0, 2) + proj_halves(wq, qt, 1, 2)
        fillers[3] = [lambda m=m: out_chunk(0, m) for m in range(KO)]
        fillers[4] = proj_halves(wq, qt, 0, 3) + proj_halves(wq, qt, 1, 3)
        fillers[5] = [lambda m=m: out_chunk(1, m) for m in range(KO)]
        fillers[6] = []
        fillers[7] = [lambda m=m: out_chunk(2, m) for m in range(KO)]
        rate = [2, 1, 1, 1, 1, 1, 1, 1]

        prev_ex = None            # stream k-1's exp tiles
        prev_ctx = None           # stream k-1's ctx psum accumulator
        prev_den = None           # den accumulator of stream k-1's n
        for k, (n, hp) in enumerate(streams):
            cur_ex = []
            cur_ctx = ps_ctx.tile([P, QW], F32, tag="ctx")
            if hp == 1:
                cur_den = ps_den.tile([P, QW], F32, tag="den")
            else:
                cur_den = prev_den
            fq = list(fillers[k])
            for kc in range(NKC):
                cur_ex.append(scores_exp(hp, n, kc))
                if prev_ex is not None:
                    pn, php = streams[k - 1]
                    ctx_pair(php, kc, prev_ex[kc], prev_ctx)
                    if hp == 1:   # pair den for both hp streams of this n
                        den4(kc, prev_ex[kc], cur_ex[kc], cur_den)
                for _ in range(rate[k]):
                    if fq:
                        fq.pop(0)()
            for f in fq:
                f()
            if prev_ex is not None:
                pn, php = streams[k - 1]
                if k == len(streams) - 1:
                    # deferred to the tail so both final den chains overlap
                    pend_norm = (pn, php, prev_ctx, cur_den)
                else:
                    # ctx(s_{k-1}) complete; den completes with this phase's
                    # den4 (php==0) or completed last phase (php==1).
                    normalize(pn, php, prev_ctx,
                              cur_den if php == 0 else prev_den)
            prev_ex, prev_ctx, prev_den = cur_ex, cur_ctx, cur_den

        # tail: both den chains' bounce DMAs fly together while the final
        # ctx runs; recips keep the DVE gap under the HAM re-throttle window
        # so out-proj(n3) stays at the warm clock.
        n, hp = streams[-1]
        pn, php, pctx, pden = pend_norm
        bc_a = den_chain_io(pn, php, pden)
        bc_b = den_chain_io(n, hp, prev_den)
        nc.vector.reciprocal_approx_fast(bc_a[:], bc_a[:])
        norm_apply(pn, php, pctx, bc_a)
        for kc in range(NKC):
            ctx_pair(hp, kc, prev_ex[kc], prev_ctx)
        nc.vector.reciprocal_approx_fast(bc_b[:], bc_b[:])
        norm_apply(n, hp, prev_ctx, bc_b)
        for m in range(KO):
            out_chunk(3, m, pool_sp=True)


def _in_maps(x, wq_f, wk_f, wv_f, wo_f):
    maps = []
    for core in range(8):
        b, g = core // 4, core % 4
        cols = slice(g * GC, (g + 1) * GC)
        maps.append({
            "xt": np.ascontiguousarray(x[b].T).astype(NP_BF16),
            "wq": np.ascontiguousarray(wq_f[:, cols]).astype(NP_BF16),
            "wk": np.ascontiguousarray(wk_f[:, cols]).astype(NP_BF16),
            "wv": np.ascontiguousarray(wv_f[:, cols]).astype(NP_BF16),
            "wo": np.ascontiguousarray(wo_f[cols, :]).astype(NP_BF16),
        })
    return maps


def _prep(x, Wq, Wk, Wv, Wo, q_scale, k_scale, v_scale, o_scale):
    x = np.asarray(x, dtype=np.float32)
    wq_f = (np.asarray(Wq).T * np.asarray(q_scale).reshape(1, -1)).astype(np.float32)
    wk_f = (np.asarray(Wk).T * np.asarray(k_scale).reshape(1, -1)).astype(np.float32)
    wv_f = (np.asarray(Wv).T * np.asarray(v_scale).reshape(1, -1)).astype(np.float32)
    wo_f = (np.asarray(Wo).T * np.asarray(o_scale).reshape(1, -1)).astype(np.float32)
    return x, _in_maps(x, wq_f, wk_f, wv_f, wo_f)


def run_traced(x, Wq, Wk, Wv, Wo, q_scale, k_scale, v_scale, o_scale):
    """Like kernel() but with NTFF tracing; returns (out, exec_time_ns, trace_path)."""
    x, maps = _prep(x, Wq, Wk, Wv, Wo, q_scale, k_scale, v_scale, o_scale)
    nc = _build()
    res = run_bass_kernel_spmd(nc, maps, core_ids=list(range(8)), trace=True)
    out = np.zeros((x.shape[0], S, D), dtype=np.float32)
    for core in range(8):
        out[core // 4] += res.results[core]["out_t"].T
    trace_path = None
    if res.instructions_and_trace is not None:
        trace_path = res.instructions_and_trace[1]
    return out, res.exec_time_ns, trace_path


def kernel(x, Wq, Wk, Wv, Wo, q_scale, k_scale, v_scale, o_scale):
    x, maps = _prep(x, Wq, Wk, Wv, Wo, q_scale, k_scale, v_scale, o_scale)
    nc = _build()
    res = run_bass_kernel_spmd(nc, maps, core_ids=list(range(8)))
    out = np.zeros((x.shape[0], S, D), dtype=np.float32)
    for core in range(8):
        out[core // 4] += res.results[core]["out_t"].T
    return out


# revision 24
# speedup vs baseline: 1.1830x; 1.1830x over previous
"""Multi-head self-attention (B=2, S=2048, D=1024, H=16) on 8 Trainium2 NeuronCores.

Sharding: batch x head-group. Core c = b*4 + g handles batch b and heads 4g..4g+3
(Megatron-style TP: Wq/Wk/Wv column-sharded, Wo row-sharded; partial outputs
summed on the host).

v3: bf16 matmuls, phase-pipelined attention. The 8 (n, hp) streams are
processed one per phase; in phase k the PE computes scores for stream k while
ctx / denominator matmuls consume stream k-1's exp tiles (held in a ring), so
the scalar engine (exp: the roofline engine at ~135us) is continuously fed and
the PE stays dense (keeps the HAM clock at 2.4 GHz). Projections and the
output projection are interleaved as per-phase fillers on their own PSUM bank.

Denominators: 4 M=1 ones-stationary col tiles (psum parts 0/32/64/96 = both
heads of both hp streams of one n) share a single 512-cycle PE slot per kc.
Normalization: denom rows -> DRAM bounce -> partition-broadcast [128,512] ->
reciprocal_approx_fast -> fused scalar_tensor_tensor (ctx * 1/den -> bf16).

PSUM: scores ring 2x[128,2,512] (4 banks) + ctx accumulators (2) + denom (1)
+ proj/out shared bank (1) = 8.
"""
import sys

sys.path.insert(0, "/opt/trn_rl_repo")

import numpy as np
import ml_dtypes

import concourse.bass as bass
import concourse.tile as tile
from concourse import bacc, mybir
from concourse.bass_utils import run_bass_kernel_spmd

F32 = mybir.dt.float32
BF16 = mybir.dt.bfloat16
NP_BF16 = ml_dtypes.bfloat16

S = 2048          # sequence length per batch
D = 1024          # embedding dim
HG = 4            # heads per core
HD = 64           # head dim
GC = HG * HD      # group cols = 256
P = 128
NQ = 4            # q chunks of 512
QW = 512          # q chunk width
NKC = 16          # key-position chunks of 128
KO = 8            # contraction chunks of 128 over D

_NC_CACHE = {}


def _build():
    if "nc" in _NC_CACHE:
        return _NC_CACHE["nc"]
    nc = bacc.Bacc(trn_type="TRN2", target_bir_lowering=False, debug=False)
    xt_d = nc.dram_tensor("xt", [D, S], BF16, kind="ExternalInput")
    wq_d = nc.dram_tensor("wq", [D, GC], BF16, kind="ExternalInput")
    wk_d = nc.dram_tensor("wk", [D, GC], BF16, kind="ExternalInput")
    wv_d = nc.dram_tensor("wv", [D, GC], BF16, kind="ExternalInput")
    wo_d = nc.dram_tensor("wo", [GC, D], BF16, kind="ExternalInput")
    out_d = nc.dram_tensor("out_t", [D, S], F32, kind="ExternalOutput")
    scr_d = nc.dram_tensor("nrm_scratch", [NQ, 2, 2, QW], F32)
    with tile.TileContext(nc) as tc:
        _emit(nc, tc, xt_d, wq_d, wk_d, wv_d, wo_d, out_d, scr_d)
    nc.compile()
    _dedup_ldweights(nc)
    _NC_CACHE["nc"] = nc
    return nc


def _dedup_ldweights(nc):
    """Drop an InstLdweights identical to the previous one when only
    (non-weight-changing) InstMatmult sit between them: the PE array keeps
    its weights, so the reload is redundant."""
    removed = 0
    for blk in nc.main_func.blocks:
        keep = []
        last = None
        for ins in blk.instructions:
            cn = type(ins).__name__
            if cn == "InstLdweights":
                si = ins.sync_info
                clean = si is None or (not si.on_wait and not si.on_update)

                def _apkey(a):
                    return (getattr(a, "memref", None),
                            getattr(a, "memsetref", None),
                            getattr(a, "offset", None),
                            str(getattr(a, "ap", None)),
                            str(getattr(a, "dtype", None)))
                key = (tuple(_apkey(a) for a in ins.ins),
                       str(getattr(ins, "tile_position", None)),
                       str(getattr(ins, "perf_mode", None)),
                       str(getattr(ins, "is_transpose", None)))
                if clean and last is not None and key == last:
                    removed += 1
                    continue
                last = key
            elif cn != "InstMatmult":
                if getattr(ins, "engine", None) is not None and                         str(getattr(ins, "engine")) == "EngineType.PE":
                    last = None
            keep.append(ins)
        blk.instructions[:] = keep
    return removed


def _emit(nc, tc, xt_d, wq_d, wk_d, wv_d, wo_d, out_d, scr_d):
    with tc.tile_pool(name="big", bufs=1) as big, \
         tc.tile_pool(name="expool", bufs=22) as expool, \
         tc.tile_pool(name="evac", bufs=4) as evac, \
         tc.tile_pool(name="nrm", bufs=4) as nrm, \
         tc.tile_pool(name="ps_sp", bufs=2, space="PSUM") as ps_sp, \
         tc.tile_pool(name="ps_ctx", bufs=2, space="PSUM") as ps_ctx, \
         tc.tile_pool(name="ps_den", bufs=1, space="PSUM") as ps_den, \
         tc.tile_pool(name="ps_o", bufs=1, space="PSUM") as ps_o:
        # ---- persistent SBUF tensors ----
        qt = big.tile([P, 2, S], BF16)        # head h at parts (h%2)*64, chunk h//2
        kt = big.tile([P, 2, S], BF16)
        va2 = big.tile([P, 2, NKC, P], BF16)  # [kpart, hp, kc, head-even|head-odd]
        ct = big.tile([P, 2, S], BF16)        # normalized ctxT, same layout as qt
        wo_sb = big.tile([P, 2, D], BF16)
        ones_w = big.tile([P, 1], BF16)       # denominator stationary
        xs = big.tile([P, KO, S], BF16)       # x.T, [d_in(128) x ko x s]
        wq = big.tile([P, KO, GC], BF16)
        wk = big.tile([P, KO, GC], BF16)
        wv = big.tile([P, KO, GC], BF16)

        nc.vector.memset(ones_w[:].bitcast(mybir.dt.uint16), 0x3F80)
        # DMA order drives time-to-first-exp: wk/wq first, then the n0 slice
        # of x, then wv (P1's V combos), the rest of x, wo last. Small pieces
        # spread across the 16 DMA queues (~22 GB/s each).
        for h in range(2):
            ks = slice(h * 4, h * 4 + 4)
            nc.sync.dma_start(
                wk[:, ks, :],
                wk_d[h * D // 2:(h + 1) * D // 2, :].rearrange(
                    "(ko p) m -> p ko m", p=P))
        for ko in range(KO):
            nc.sync.dma_start(xs[:, ko, 0:QW], xt_d[ko * P:(ko + 1) * P, 0:QW])
        for h in range(2):
            ks = slice(h * 4, h * 4 + 4)
            nc.sync.dma_start(
                wq[:, ks, :],
                wq_d[h * D // 2:(h + 1) * D // 2, :].rearrange(
                    "(ko p) m -> p ko m", p=P))
        for h in range(2):
            ks = slice(h * 4, h * 4 + 4)
            nc.sync.dma_start(
                wv[:, ks, :],
                wv_d[h * D // 2:(h + 1) * D // 2, :].rearrange(
                    "(ko p) m -> p ko m", p=P))
        for n in range(1, NQ):
            for ko in range(KO):
                nc.sync.dma_start(xs[:, ko, n * QW:(n + 1) * QW],
                                  xt_d[ko * P:(ko + 1) * P, n * QW:(n + 1) * QW])
        for c in range(4):
            nc.sync.dma_start(
                wo_sb[:, c // 2, (c % 2) * QW:(c % 2 + 1) * QW],
                wo_d[(c // 2) * P:(c // 2 + 1) * P,
                     (c % 2) * QW:(c % 2 + 1) * QW])

        def proj_combo(w_sb, dst, m, n):
            """dst[:, m, n*QW:+QW] = (w_sb chunk).T @ xs chunk, K=128 x 8."""
            pp = ps_sp.tile([P, 2, QW], F32, tag="sp")
            for ko in range(KO):
                nc.tensor.matmul(pp[:, 0, :],
                                 w_sb[:, ko, m * P:(m + 1) * P],
                                 xs[:, ko, n * QW:(n + 1) * QW],
                                 start=(ko == 0), stop=(ko == KO - 1))
            nc.vector.tensor_copy(dst[:, m, n * QW:(n + 1) * QW], pp[:, 0, :])

        def v_combo(sc):
            """va2[:, :, sc, :] = V rows sc*128..+128 (natural layout)."""
            pp = ps_sp.tile([P, 2, QW], F32, tag="sp")
            for ko in range(KO):
                nc.tensor.matmul(pp[:, 0, 0:GC],
                                 xs[:, ko, sc * P:(sc + 1) * P],
                                 wv[:, ko, :],
                                 start=(ko == 0), stop=(ko == KO - 1))
            nc.vector.tensor_copy(
                va2[:, :, sc, :],
                pp[:, 0, 0:GC].rearrange("p (h c) -> p h c", c=P))

        def scores_exp(hp, n, kc):
            """Returns ex tile [P, 2, QW] bf16 = exp(scoresT/8) for both heads."""
            sp = ps_sp.tile([P, 2, QW], F32, tag="sp")
            for e in range(2):
                lo = e * 64
                nc.tensor.matmul(
                    sp[:, e, :],
                    kt[lo:lo + 64, hp, kc * P:(kc + 1) * P],
                    qt[lo:lo + 64, hp, n * QW:(n + 1) * QW],
                    start=True, stop=True)
            ex = expool.tile([P, 2, QW], BF16, tag="ex")
            nc.scalar.activation(
                ex[:].rearrange("p a b -> p (a b)"),
                sp[:].rearrange("p a b -> p (a b)"),
                mybir.ActivationFunctionType.Exp,
                scale=0.125)
            return ex

        def ctx_pair(hp, kc, ex, ctx_ps):
            first, last = kc == 0, kc == NKC - 1
            nc.tensor.matmul(ctx_ps[0:64, :], va2[:, hp, kc, 0:64],
                             ex[:, 0, :], start=first, stop=last,
                             tile_position=(0, 0))
            nc.tensor.matmul(ctx_ps[64:128, :], va2[:, hp, kc, 64:128],
                             ex[:, 1, :], start=first, stop=last,
                             tile_position=(0, 64))

        def den4(kc, ex0, ex1, den_ps):
            """One slot: denominators for both heads of both hp streams of n."""
            first, last = kc == 0, kc == NKC - 1
            for j, exs in enumerate((ex0[:, 0, :], ex0[:, 1, :],
                                     ex1[:, 0, :], ex1[:, 1, :])):
                c = 32 * j
                nc.tensor.matmul(den_ps[c:c + 1, :], ones_w[:, 0:1], exs,
                                 start=first, stop=last, tile_position=(0, c))

        def den_chain_io(n, hp, den_ps):
            """DRAM bounce for (n, hp) denoms -> bc broadcast tile (no recip)."""
            base = 64 * hp
            dsb = nrm.tile([P, QW], F32, tag="dsb")
            for e in range(2):
                r = base + 32 * e
                nc.vector.tensor_copy(dsb[r:r + 1, :], den_ps[r:r + 1, :])
            for e in range(2):
                r = base + 32 * e
                sl = scr_d[n, hp, e]
                nc.sync.dma_start(sl.unsqueeze(0), dsb[r:r + 1, :])
            bc = nrm.tile([P, QW], F32, tag="bc")
            for e in range(2):
                sl = scr_d[n, hp, e]
                bc_src = bass.AP(tensor=sl.tensor, offset=sl.offset,
                                 ap=[[0, 64]] + list(sl.ap))
                nc.sync.dma_start(bc[64 * e:64 * (e + 1), :], bc_src)
            return bc

        def den_chain(n, hp, den_ps):
            bc = den_chain_io(n, hp, den_ps)
            nc.vector.reciprocal_approx_fast(bc[:], bc[:])
            return bc

        def norm_apply(n, hp, ctx_ps, bc):
            ns = slice(n * QW, (n + 1) * QW)
            nc.vector.scalar_tensor_tensor(
                ct[:, hp, ns], ctx_ps[:], 1.0, bc[:],
                mybir.AluOpType.mult, mybir.AluOpType.mult)

        def normalize(n, hp, ctx_ps, den_ps):
            norm_apply(n, hp, ctx_ps, den_chain(n, hp, den_ps))

        def out_chunk(n, m, pool_sp=False):
            ns = slice(n * QW, (n + 1) * QW)
            if pool_sp:
                pot = ps_sp.tile([P, 2, QW], F32, tag="sp", name=f"po_sp_{n}_{m}")
                po = pot[:, 0, :]
            else:
                pot = ps_o.tile([P, QW], F32, tag="po", name=f"po_{n}_{m}")
                po = pot[:]
            nc.tensor.matmul(po, wo_sb[:, 0, m * P:(m + 1) * P],
                             ct[:, 0, ns], start=True, stop=False)
            nc.tensor.matmul(po, wo_sb[:, 1, m * P:(m + 1) * P],
                             ct[:, 1, ns], start=False, stop=True)
            ot = evac.tile([P, QW], F32, tag="ot")
            nc.vector.tensor_copy(ot[:], po)
            nc.sync.dma_start(out_d[m * P:(m + 1) * P, ns], ot[:])

        def proj_halves(w_sb, dst, m, n):
            """proj_combo split in two filler units (smaller PE injections)."""
            cell = {}

            def f1():
                cell["pp"] = ps_sp.tile([P, 2, QW], F32, tag="sp", name=f"ph_{id(cell)}")
                for ko in range(4):
                    nc.tensor.matmul(cell["pp"][:, 0, :],
                                     w_sb[:, ko, m * P:(m + 1) * P],
                                     xs[:, ko, n * QW:(n + 1) * QW],
                                     start=(ko == 0), stop=False)

            def f2():
                pp = cell["pp"]
                for ko in range(4, KO):
                    nc.tensor.matmul(pp[:, 0, :],
                                     w_sb[:, ko, m * P:(m + 1) * P],
                                     xs[:, ko, n * QW:(n + 1) * QW],
                                     start=False, stop=(ko == KO - 1))
                nc.vector.tensor_copy(dst[:, m, n * QW:(n + 1) * QW],
                                      pp[:, 0, :])

            return [f1, f2]

        # ---- schedule: emission order == per-engine execution order ----
        streams = [(n, hp) for n in range(NQ) for hp in range(2)]

        # PE clock warm-up: dummy matmuls while the input DMAs run, so the
        # HAM un-throttles the PE (1.2 -> 2.4 GHz) before real work.
        dmy_ps = ps_o.tile([P, QW], F32, tag="po", name="dmy_ps")
        for _ in range(170):
            nc.tensor.matmul(dmy_ps[0:1, 0:1], ones_w[:, 0:1], ones_w[:, 0:1],
                             start=True, stop=True)

        # P0 (dense warm-up): minimum to unblock stream 0 = (n0, h0)
        proj_combo(wk, kt, 0, 0)
        proj_combo(wq, qt, 0, 0)

        # per-phase fillers; pops-per-kc in fillers_rate
        fillers = [[] for _ in range(8)]
        # P1: V (ctx(s0) needs it in P2), rest of kt, qt(m1, n0) for s1.
        # KT m1 / QT m1n0 are needed at P2 kc0 -> interleave them early.
        p1 = []
        extras = [lambda: proj_combo(wk, kt, 0, 1),
                  lambda: proj_combo(wk, kt, 1, 0),
                  lambda: proj_combo(wk, kt, 0, 2),
                  lambda: proj_combo(wk, kt, 1, 1),
                  lambda: proj_combo(wk, kt, 0, 3),
                  lambda: proj_combo(wk, kt, 1, 2),
                  lambda: proj_combo(wk, kt, 1, 3),
                  lambda: proj_combo(wq, qt, 1, 0)]
        for sc in range(NKC):
            p1.append(lambda sc=sc: v_combo(sc))
            if sc < len(extras):
                p1.append(extras[sc])
        fillers[0] = p1
        fillers[1] = proj_halves(wq, qt, 0, 1) + proj_halves(wq, qt, 1, 1)
        fillers[2] = proj_halves(wq, qt, 0, 2) + proj_halves(wq, qt, 1, 2)
        fillers[3] = [lambda m=m: out_chunk(0, m) for m in range(KO)]
        fillers[4] = proj_halves(wq, qt, 0, 3) + proj_halves(wq, qt, 1, 3)
        fillers[5] = [lambda m=m: out_chunk(1, m) for m in range(KO)]
        fillers[6] = []
        fillers[7] = [lambda m=m: out_chunk(2, m) for m in range(KO)]
        rate = [2, 1, 1, 1, 1, 1, 1, 1]

        prev_ex = None            # stream k-1's exp tiles
        prev_ctx = None           # stream k-1's ctx psum accumulator
        prev_den = None           # den accumulator of stream k-1's n
        for k, (n, hp) in enumerate(streams):
            cur_ex = []
            cur_ctx = ps_ctx.tile([P, QW], F32, tag="ctx")
            if hp == 1:
                cur_den = ps_den.tile([P, QW], F32, tag="den")
            else:
                cur_den = prev_den
            fq = list(fillers[k])
            for kc in range(NKC):
                cur_ex.append(scores_exp(hp, n, kc))
                if prev_ex is not None:
                    pn, php = streams[k - 1]
                    ctx_pair(php, kc, prev_ex[kc], prev_ctx)
                    if hp == 1:   # pair den for both hp streams of this n
                        den4(kc, prev_ex[kc], cur_ex[kc], cur_den)
                for _ in range(rate[k]):
                    if fq:
                        fq.pop(0)()
            for f in fq:
                f()
            if prev_ex is not None:
                pn, php = streams[k - 1]
                if k == len(streams) - 1:
                    # deferred to the tail so both final den chains overlap
                    pend_norm = (pn, php, prev_ctx, cur_den)
                else:
                    # ctx(s_{k-1}) complete; den completes with this phase's
                    # den4 (php==0) or completed last phase (php==1).
                    normalize(pn, php, prev_ctx,
                              cur_den if php == 0 else prev_den)
            prev_ex, prev_ctx, prev_den = cur_ex, cur_ctx, cur_den

        # tail: both den chains' bounce DMAs fly together while the final
        # ctx runs; recips keep the DVE gap under the HAM re-throttle window
        # so out-proj(n3) stays at the warm clock.
        n, hp = streams[-1]
        pn, php, pctx, pden = pend_norm
        bc_a = den_chain_io(pn, php, pden)
        bc_b = den_chain_io(n, hp, prev_den)
        nc.vector.reciprocal_approx_fast(bc_a[:], bc_a[:])
        norm_apply(pn, php, pctx, bc_a)
        for kc in range(NKC):
            ctx_pair(hp, kc, prev_ex[kc], prev_ctx)
        nc.vector.reciprocal_approx_fast(bc_b[:], bc_b[:])
        norm_apply(n, hp, prev_ctx, bc_b)
        for m in range(KO):
            out_chunk(3, m, pool_sp=True)


def _in_maps(x, wq_f, wk_f, wv_f, wo_f):
    maps = []
    for core in range(8):
        b, g = core // 4, core % 4
        cols = slice(g * GC, (g + 1) * GC)
        maps.append({
            "xt": np.ascontiguousarray(x[b].T).astype(NP_BF16),
            "wq": np.ascontiguousarray(wq_f[:, cols]).astype(NP_BF16),
            "wk": np.ascontiguousarray(wk_f[:, cols]).astype(NP_BF16),
            "wv": np.ascontiguousarray(wv_f[:, cols]).astype(NP_BF16),
            "wo": np.ascontiguousarray(wo_f[cols, :]).astype(NP_BF16),
        })
    return maps


def _prep(x, Wq, Wk, Wv, Wo, q_scale, k_scale, v_scale, o_scale):
    x = np.asarray(x, dtype=np.float32)
    wq_f = (np.asarray(Wq).T * np.asarray(q_scale).reshape(1, -1)).astype(np.float32)
    wk_f = (np.asarray(Wk).T * np.asarray(k_scale).reshape(1, -1)).astype(np.float32)
    wv_f = (np.asarray(Wv).T * np.asarray(v_scale).reshape(1, -1)).astype(np.float32)
    wo_f = (np.asarray(Wo).T * np.asarray(o_scale).reshape(1, -1)).astype(np.float32)
    return x, _in_maps(x, wq_f, wk_f, wv_f, wo_f)


def run_traced(x, Wq, Wk, Wv, Wo, q_scale, k_scale, v_scale, o_scale):
    """Like kernel() but with NTFF tracing; returns (out, exec_time_ns, trace_path)."""
    x, maps = _prep(x, Wq, Wk, Wv, Wo, q_scale, k_scale, v_scale, o_scale)
    nc = _build()
    res = run_bass_kernel_spmd(nc, maps, core_ids=list(range(8)), trace=True)
    out = np.zeros((x.shape[0], S, D), dtype=np.float32)
    for core in range(8):
        out[core // 4] += res.results[core]["out_t"].T
    trace_path = None
    if res.instructions_and_trace is not None:
        trace_path = res.instructions_and_trace[1]
    return out, res.exec_time_ns, trace_path


def kernel(x, Wq, Wk, Wv, Wo, q_scale, k_scale, v_scale, o_scale):
    x, maps = _prep(x, Wq, Wk, Wv, Wo, q_scale, k_scale, v_scale, o_scale)
    nc = _build()
    res = run_bass_kernel_spmd(nc, maps, core_ids=list(range(8)))
    out = np.zeros((x.shape[0], S, D), dtype=np.float32)
    for core in range(8):
        out[core // 4] += res.results[core]["out_t"].T
    return out


# revision 27
# speedup vs baseline: 1.1924x; 1.0079x over previous
"""Multi-head self-attention (B=2, S=2048, D=1024, H=16) on 8 Trainium2 NeuronCores.

Sharding: batch x head-group. Core c = b*4 + g handles batch b and heads 4g..4g+3
(Megatron-style TP: Wq/Wk/Wv column-sharded, Wo row-sharded; partial outputs
summed on the host).

v3: bf16 matmuls, phase-pipelined attention. The 8 (n, hp) streams are
processed one per phase; in phase k the PE computes scores for stream k while
ctx / denominator matmuls consume stream k-1's exp tiles (held in a ring), so
the scalar engine (exp: the roofline engine at ~135us) is continuously fed and
the PE stays dense (keeps the HAM clock at 2.4 GHz). Projections and the
output projection are interleaved as per-phase fillers on their own PSUM bank.

Denominators: 4 M=1 ones-stationary col tiles (psum parts 0/32/64/96 = both
heads of both hp streams of one n) share a single 512-cycle PE slot per kc.
Normalization: denom rows -> DRAM bounce -> partition-broadcast [128,512] ->
reciprocal_approx_fast -> fused scalar_tensor_tensor (ctx * 1/den -> bf16).

PSUM: scores ring 2x[128,2,512] (4 banks) + ctx accumulators (2) + denom (1)
+ proj/out shared bank (1) = 8.
"""
import sys

sys.path.insert(0, "/opt/trn_rl_repo")

import numpy as np
import ml_dtypes

import concourse.bass as bass
import concourse.tile as tile
from concourse import bacc, mybir
from concourse.bass_utils import run_bass_kernel_spmd

F32 = mybir.dt.float32
BF16 = mybir.dt.bfloat16
NP_BF16 = ml_dtypes.bfloat16

S = 2048          # sequence length per batch
D = 1024          # embedding dim
HG = 4            # heads per core
HD = 64           # head dim
GC = HG * HD      # group cols = 256
P = 128
NQ = 4            # q chunks of 512
QW = 512          # q chunk width
NKC = 16          # key-position chunks of 128
KO = 8            # contraction chunks of 128 over D

_NC_CACHE = {}


def _build():
    if "nc" in _NC_CACHE:
        return _NC_CACHE["nc"]
    nc = bacc.Bacc(trn_type="TRN2", target_bir_lowering=False, debug=False)
    xt_d = nc.dram_tensor("xt", [D, S], BF16, kind="ExternalInput")
    wq_d = nc.dram_tensor("wq", [D, GC], BF16, kind="ExternalInput")
    wk_d = nc.dram_tensor("wk", [D, GC], BF16, kind="ExternalInput")
    wv_d = nc.dram_tensor("wv", [D, GC], BF16, kind="ExternalInput")
    wo_d = nc.dram_tensor("wo", [GC, D], BF16, kind="ExternalInput")
    out_d = nc.dram_tensor("out_t", [D, S], F32, kind="ExternalOutput")
    scr_d = nc.dram_tensor("nrm_scratch", [NQ, 2, 2, QW], F32)
    with tile.TileContext(nc) as tc:
        _emit(nc, tc, xt_d, wq_d, wk_d, wv_d, wo_d, out_d, scr_d)
    nc.compile()
    _dedup_ldweights(nc)
    _NC_CACHE["nc"] = nc
    return nc


def _dedup_ldweights(nc):
    """Drop an InstLdweights identical to the previous one when only
    (non-weight-changing) InstMatmult sit between them: the PE array keeps
    its weights, so the reload is redundant."""
    removed = 0
    for blk in nc.main_func.blocks:
        keep = []
        last = None
        for ins in blk.instructions:
            cn = type(ins).__name__
            if cn == "InstLdweights":
                si = ins.sync_info
                clean = si is None or (not si.on_wait and not si.on_update)

                def _apkey(a):
                    return (getattr(a, "memref", None),
                            getattr(a, "memsetref", None),
                            getattr(a, "offset", None),
                            str(getattr(a, "ap", None)),
                            str(getattr(a, "dtype", None)))
                key = (tuple(_apkey(a) for a in ins.ins),
                       str(getattr(ins, "tile_position", None)),
                       str(getattr(ins, "perf_mode", None)),
                       str(getattr(ins, "is_transpose", None)))
                if clean and last is not None and key == last:
                    removed += 1
                    continue
                last = key
            elif cn != "InstMatmult":
                if getattr(ins, "engine", None) is not None and                         str(getattr(ins, "engine")) == "EngineType.PE":
                    last = None
            keep.append(ins)
        blk.instructions[:] = keep
    return removed


def _emit(nc, tc, xt_d, wq_d, wk_d, wv_d, wo_d, out_d, scr_d):
    with tc.tile_pool(name="big", bufs=1) as big, \
         tc.tile_pool(name="expool", bufs=22) as expool, \
         tc.tile_pool(name="evac", bufs=4) as evac, \
         tc.tile_pool(name="nrm", bufs=4) as nrm, \
         tc.tile_pool(name="ps_sp", bufs=2, space="PSUM") as ps_sp, \
         tc.tile_pool(name="ps_ctx", bufs=2, space="PSUM") as ps_ctx, \
         tc.tile_pool(name="ps_den", bufs=1, space="PSUM") as ps_den, \
         tc.tile_pool(name="ps_o", bufs=1, space="PSUM") as ps_o:
        # ---- persistent SBUF tensors ----
        qt = big.tile([P, 2, S], BF16)        # head h at parts (h%2)*64, chunk h//2
        kt = big.tile([P, 2, S], BF16)
        va2 = big.tile([P, 2, NKC, P], BF16)  # [kpart, hp, kc, head-even|head-odd]
        ct = big.tile([P, 2, S], BF16)        # normalized ctxT, same layout as qt
        wo_sb = big.tile([P, 2, D], BF16)
        ones_w = big.tile([P, 1], BF16)       # denominator stationary
        ones64 = big.tile([P, 64], BF16)      # K=1 broadcast stationary
        xs = big.tile([P, KO, S], BF16)       # x.T, [d_in(128) x ko x s]
        wq = big.tile([P, KO, GC], BF16)
        wk = big.tile([P, KO, GC], BF16)
        wv = big.tile([P, KO, GC], BF16)

        nc.vector.memset(ones_w[:].bitcast(mybir.dt.uint16), 0x3F80)
        nc.vector.memset(ones64[:].bitcast(mybir.dt.uint16), 0x3F80)
        # DMA order drives time-to-first-exp: wk/wq first, then the n0 slice
        # of x, then wv (P1's V combos), the rest of x, wo last. Small pieces
        # spread across the 16 DMA queues (~22 GB/s each).
        for h in range(2):
            ks = slice(h * 4, h * 4 + 4)
            nc.sync.dma_start(
                wk[:, ks, :],
                wk_d[h * D // 2:(h + 1) * D // 2, :].rearrange(
                    "(ko p) m -> p ko m", p=P))
        for ko in range(KO):
            nc.sync.dma_start(xs[:, ko, 0:QW], xt_d[ko * P:(ko + 1) * P, 0:QW])
        for h in range(2):
            ks = slice(h * 4, h * 4 + 4)
            nc.sync.dma_start(
                wq[:, ks, :],
                wq_d[h * D // 2:(h + 1) * D // 2, :].rearrange(
                    "(ko p) m -> p ko m", p=P))
        for h in range(2):
            ks = slice(h * 4, h * 4 + 4)
            nc.sync.dma_start(
                wv[:, ks, :],
                wv_d[h * D // 2:(h + 1) * D // 2, :].rearrange(
                    "(ko p) m -> p ko m", p=P))
        for n in range(1, NQ):
            for ko in range(KO):
                nc.sync.dma_start(xs[:, ko, n * QW:(n + 1) * QW],
                                  xt_d[ko * P:(ko + 1) * P, n * QW:(n + 1) * QW])
        for c in range(4):
            nc.sync.dma_start(
                wo_sb[:, c // 2, (c % 2) * QW:(c % 2 + 1) * QW],
                wo_d[(c // 2) * P:(c // 2 + 1) * P,
                     (c % 2) * QW:(c % 2 + 1) * QW])

        def proj_combo(w_sb, dst, m, n):
            """dst[:, m, n*QW:+QW] = (w_sb chunk).T @ xs chunk, K=128 x 8."""
            pp = ps_sp.tile([P, 2, QW], F32, tag="sp")
            for ko in range(KO):
                nc.tensor.matmul(pp[:, 0, :],
                                 w_sb[:, ko, m * P:(m + 1) * P],
                                 xs[:, ko, n * QW:(n + 1) * QW],
                                 start=(ko == 0), stop=(ko == KO - 1))
            nc.vector.tensor_copy(dst[:, m, n * QW:(n + 1) * QW], pp[:, 0, :])

        def v_combo(sc):
            """va2[:, :, sc, :] = V rows sc*128..+128 (natural layout)."""
            pp = ps_sp.tile([P, 2, QW], F32, tag="sp")
            for ko in range(KO):
                nc.tensor.matmul(pp[:, 0, 0:GC],
                                 xs[:, ko, sc * P:(sc + 1) * P],
                                 wv[:, ko, :],
                                 start=(ko == 0), stop=(ko == KO - 1))
            nc.vector.tensor_copy(
                va2[:, :, sc, :],
                pp[:, 0, 0:GC].rearrange("p (h c) -> p h c", c=P))

        def scores_exp(hp, n, kc):
            """Returns ex tile [P, 2, QW] bf16 = exp(scoresT/8) for both heads."""
            sp = ps_sp.tile([P, 2, QW], F32, tag="sp")
            for e in range(2):
                lo = e * 64
                nc.tensor.matmul(
                    sp[:, e, :],
                    kt[lo:lo + 64, hp, kc * P:(kc + 1) * P],
                    qt[lo:lo + 64, hp, n * QW:(n + 1) * QW],
                    start=True, stop=True)
            ex = expool.tile([P, 2, QW], BF16, tag="ex")
            nc.scalar.activation(
                ex[:].rearrange("p a b -> p (a b)"),
                sp[:].rearrange("p a b -> p (a b)"),
                mybir.ActivationFunctionType.Exp,
                scale=0.125)
            return ex

        def ctx_pair(hp, kc, ex, ctx_ps):
            first, last = kc == 0, kc == NKC - 1
            nc.tensor.matmul(ctx_ps[0:64, :], va2[:, hp, kc, 0:64],
                             ex[:, 0, :], start=first, stop=last,
                             tile_position=(0, 0))
            nc.tensor.matmul(ctx_ps[64:128, :], va2[:, hp, kc, 64:128],
                             ex[:, 1, :], start=first, stop=last,
                             tile_position=(0, 64))

        def den4(kc, ex0, ex1, den_ps):
            """One slot: denominators for both heads of both hp streams of n."""
            first, last = kc == 0, kc == NKC - 1
            for j, exs in enumerate((ex0[:, 0, :], ex0[:, 1, :],
                                     ex1[:, 0, :], ex1[:, 1, :])):
                c = 32 * j
                nc.tensor.matmul(den_ps[c:c + 1, :], ones_w[:, 0:1], exs,
                                 start=first, stop=last, tile_position=(0, c))

        def den_chain_io(n, hp, den_ps):
            """DRAM bounce for (n, hp) denoms -> bc broadcast tile (no recip)."""
            base = 64 * hp
            dsb = nrm.tile([P, QW], F32, tag="dsb")
            for e in range(2):
                r = base + 32 * e
                nc.vector.tensor_copy(dsb[r:r + 1, :], den_ps[r:r + 1, :])
            for e in range(2):
                r = base + 32 * e
                sl = scr_d[n, hp, e]
                nc.sync.dma_start(sl.unsqueeze(0), dsb[r:r + 1, :])
            bc = nrm.tile([P, QW], F32, tag="bc")
            for e in range(2):
                sl = scr_d[n, hp, e]
                bc_src = bass.AP(tensor=sl.tensor, offset=sl.offset,
                                 ap=[[0, 64]] + list(sl.ap))
                nc.sync.dma_start(bc[64 * e:64 * (e + 1), :], bc_src)
            return bc

        def den_chain(n, hp, den_ps):
            bc = den_chain_io(n, hp, den_ps)
            nc.vector.reciprocal_approx_fast(bc[:], bc[:])
            return bc

        def norm_apply(n, hp, ctx_ps, bc):
            ns = slice(n * QW, (n + 1) * QW)
            nc.vector.scalar_tensor_tensor(
                ct[:, hp, ns], ctx_ps[:], 1.0, bc[:],
                mybir.AluOpType.mult, mybir.AluOpType.mult)

        def normalize(n, hp, ctx_ps, den_ps):
            norm_apply(n, hp, ctx_ps, den_chain(n, hp, den_ps))

        def out_chunk(n, m, pool_sp=False):
            ns = slice(n * QW, (n + 1) * QW)
            if pool_sp:
                pot = ps_sp.tile([P, 2, QW], F32, tag="sp", name=f"po_sp_{n}_{m}")
                po = pot[:, 0, :]
            else:
                pot = ps_o.tile([P, QW], F32, tag="po", name=f"po_{n}_{m}")
                po = pot[:]
            nc.tensor.matmul(po, wo_sb[:, 0, m * P:(m + 1) * P],
                             ct[:, 0, ns], start=True, stop=False)
            nc.tensor.matmul(po, wo_sb[:, 1, m * P:(m + 1) * P],
                             ct[:, 1, ns], start=False, stop=True)
            ot = evac.tile([P, QW], F32, tag="ot")
            nc.vector.tensor_copy(ot[:], po)
            nc.sync.dma_start(out_d[m * P:(m + 1) * P, ns], ot[:])

        def proj_halves(w_sb, dst, m, n):
            """proj_combo split in two filler units (smaller PE injections)."""
            cell = {}

            def f1():
                cell["pp"] = ps_sp.tile([P, 2, QW], F32, tag="sp", name=f"ph_{id(cell)}")
                for ko in range(4):
                    nc.tensor.matmul(cell["pp"][:, 0, :],
                                     w_sb[:, ko, m * P:(m + 1) * P],
                                     xs[:, ko, n * QW:(n + 1) * QW],
                                     start=(ko == 0), stop=False)

            def f2():
                pp = cell["pp"]
                for ko in range(4, KO):
                    nc.tensor.matmul(pp[:, 0, :],
                                     w_sb[:, ko, m * P:(m + 1) * P],
                                     xs[:, ko, n * QW:(n + 1) * QW],
                                     start=False, stop=(ko == KO - 1))
                nc.vector.tensor_copy(dst[:, m, n * QW:(n + 1) * QW],
                                      pp[:, 0, :])

            return [f1, f2]

        # ---- schedule: emission order == per-engine execution order ----
        streams = [(n, hp) for n in range(NQ) for hp in range(2)]

        # PE clock warm-up: dummy matmuls while the input DMAs run, so the
        # HAM un-throttles the PE (1.2 -> 2.4 GHz) before real work.
        dmy_ps = ps_o.tile([P, QW], F32, tag="po", name="dmy_ps")
        for _ in range(170):
            nc.tensor.matmul(dmy_ps[0:1, 0:1], ones_w[:, 0:1], ones_w[:, 0:1],
                             start=True, stop=True)

        # P0 (dense warm-up): minimum to unblock stream 0 = (n0, h0)
        proj_combo(wk, kt, 0, 0)
        proj_combo(wq, qt, 0, 0)

        # per-phase fillers; pops-per-kc in fillers_rate
        fillers = [[] for _ in range(8)]
        # P1: V (ctx(s0) needs it in P2), rest of kt, qt(m1, n0) for s1.
        # KT m1 / QT m1n0 are needed at P2 kc0 -> interleave them early.
        p1 = []
        extras = [lambda: proj_combo(wk, kt, 0, 1),
                  lambda: proj_combo(wk, kt, 1, 0),
                  lambda: proj_combo(wk, kt, 0, 2),
                  lambda: proj_combo(wk, kt, 1, 1),
                  lambda: proj_combo(wk, kt, 0, 3),
                  lambda: proj_combo(wk, kt, 1, 2),
                  lambda: proj_combo(wk, kt, 1, 3),
                  lambda: proj_combo(wq, qt, 1, 0)]
        for sc in range(NKC):
            p1.append(lambda sc=sc: v_combo(sc))
            if sc < len(extras):
                p1.append(extras[sc])
        fillers[0] = p1
        fillers[1] = proj_halves(wq, qt, 0, 1) + proj_halves(wq, qt, 1, 1)
        fillers[2] = proj_halves(wq, qt, 0, 2) + proj_halves(wq, qt, 1, 2)
        fillers[3] = [lambda m=m: out_chunk(0, m) for m in range(KO)]
        fillers[4] = proj_halves(wq, qt, 0, 3) + proj_halves(wq, qt, 1, 3)
        fillers[5] = [lambda m=m: out_chunk(1, m) for m in range(KO)]
        fillers[6] = []
        fillers[7] = [lambda m=m: out_chunk(2, m) for m in range(KO)]
        rate = [2, 1, 1, 1, 1, 1, 1, 1]

        prev_ex = None            # stream k-1's exp tiles
        prev_ctx = None           # stream k-1's ctx psum accumulator
        prev_den = None           # den accumulator of stream k-1's n
        for k, (n, hp) in enumerate(streams):
            cur_ex = []
            cur_ctx = ps_ctx.tile([P, QW], F32, tag="ctx")
            if hp == 1:
                cur_den = ps_den.tile([P, QW], F32, tag="den")
            else:
                cur_den = prev_den
            fq = list(fillers[k])
            for kc in range(NKC):
                cur_ex.append(scores_exp(hp, n, kc))
                if prev_ex is not None:
                    pn, php = streams[k - 1]
                    ctx_pair(php, kc, prev_ex[kc], prev_ctx)
                    if hp == 1:   # pair den for both hp streams of this n
                        den4(kc, prev_ex[kc], cur_ex[kc], cur_den)
                for _ in range(rate[k]):
                    if fq:
                        fq.pop(0)()
            for f in fq:
                f()
            if prev_ex is not None:
                pn, php = streams[k - 1]
                if k == len(streams) - 1:
                    # deferred to the tail so both final den chains overlap
                    pend_norm = (pn, php, prev_ctx, cur_den)
                else:
                    # ctx(s_{k-1}) complete; den completes with this phase's
                    # den4 (php==0) or completed last phase (php==1).
                    normalize(pn, php, prev_ctx,
                              cur_den if php == 0 else prev_den)
            prev_ex, prev_ctx, prev_den = cur_ex, cur_ctx, cur_den

        # tail: normalize via K=1 PE broadcast matmuls (no DRAM bounce, no
        # sync-queue DMA serialization; the PE stays warm into out-proj(n3)).
        n, hp = streams[-1]
        pn, php, pctx, pden = pend_norm
        dsb_t = nrm.tile([P, QW], BF16, tag="dsbt", name="dsb_t")
        for r in (0, 32, 64, 96):
            nc.vector.tensor_copy(dsb_t[r:r + 1, :], pden[r:r + 1, :])
        for kc in range(NKC):
            ctx_pair(hp, kc, prev_ex[kc], prev_ctx)
        bcps = []
        for j, base in enumerate((0, 64)):      # hp0 rows {0,32}, hp1 {64,96}
            bcp = ps_sp.tile([P, 2, QW], F32, tag="sp", name=f"bcp{j}")
            for e in range(2):
                r = base + 32 * e
                nc.tensor.matmul(bcp[64 * e:64 * (e + 1), 0, :],
                                 ones64[r:r + 1, :], dsb_t[r:r + 1, :],
                                 start=True, stop=True,
                                 tile_position=(r, 64 * e))
            bcps.append(bcp)
        for (pnn, pph, pcx), bcp in zip(((pn, php, pctx), (n, hp, prev_ctx)),
                                        bcps):
            bc = nrm.tile([P, QW], F32, tag="bc")
            nc.vector.reciprocal_approx_fast(bc[:], bcp[:, 0, :])
            norm_apply(pnn, pph, pcx, bc)
        for m in range(KO):
            out_chunk(3, m, pool_sp=True)


def _in_maps(x, wq_f, wk_f, wv_f, wo_f):
    maps = []
    for core in range(8):
        b, g = core // 4, core % 4
        cols = slice(g * GC, (g + 1) * GC)
        maps.append({
            "xt": np.ascontiguousarray(x[b].T).astype(NP_BF16),
            "wq": np.ascontiguousarray(wq_f[:, cols]).astype(NP_BF16),
            "wk": np.ascontiguousarray(wk_f[:, cols]).astype(NP_BF16),
            "wv": np.ascontiguousarray(wv_f[:, cols]).astype(NP_BF16),
            "wo": np.ascontiguousarray(wo_f[cols, :]).astype(NP_BF16),
        })
    return maps


def _prep(x, Wq, Wk, Wv, Wo, q_scale, k_scale, v_scale, o_scale):
    x = np.asarray(x, dtype=np.float32)
    wq_f = (np.asarray(Wq).T * np.asarray(q_scale).reshape(1, -1)).astype(np.float32)
    wk_f = (np.asarray(Wk).T * np.asarray(k_scale).reshape(1, -1)).astype(np.float32)
    wv_f = (np.asarray(Wv).T * np.asarray(v_scale).reshape(1, -1)).astype(np.float32)
    wo_f = (np.asarray(Wo).T * np.asarray(o_scale).reshape(1, -1)).astype(np.float32)
    return x, _in_maps(x, wq_f, wk_f, wv_f, wo_f)


def run_traced(x, Wq, Wk, Wv, Wo, q_scale, k_scale, v_scale, o_scale):
    """Like kernel() but with NTFF tracing; returns (out, exec_time_ns, trace_path)."""
    x, maps = _prep(x, Wq, Wk, Wv, Wo, q_scale, k_scale, v_scale, o_scale)
    nc = _build()
    res = run_bass_kernel_spmd(nc, maps, core_ids=list(range(8)), trace=True)
    out = np.zeros((x.shape[0], S, D), dtype=np.float32)
    for core in range(8):
        out[core // 4] += res.results[core]["out_t"].T
    trace_path = None
    if res.instructions_and_trace is not None:
        trace_path = res.instructions_and_trace[1]
    return out, res.exec_time_ns, trace_path


def kernel(x, Wq, Wk, Wv, Wo, q_scale, k_scale, v_scale, o_scale):
    x, maps = _prep(x, Wq, Wk, Wv, Wo, q_scale, k_scale, v_scale, o_scale)
    nc = _build()
    res = run_bass_kernel_spmd(nc, maps, core_ids=list(range(8)))
    out = np.zeros((x.shape[0], S, D), dtype=np.float32)
    for core in range(8):
        out[core // 4] += res.results[core]["out_t"].T
    return out
